# revision 19
# baseline (speedup 1.0000x reference)
"""Trainium2 Bass kernel for GroupedQueryAttention (sparse sliding-window + global).

Sharding: 8 cores = 2 (batch) x 4 (GQA groups). Core c handles batch c//4 and
kv-head g=c%4 together with its 4 query heads (heads 4g..4g+3). Wq/Wk/Wv are
column-sharded, Wo row-sharded; each core emits a transposed partial output
outT = (context_g @ Wo_g)^T (bf16) which the host transposes and sums per batch.

v3 design notes:
- host passes x pre-transposed (xT, bf16): the QKV matmul consumes xT chunks as
  lhsT directly -> no on-device x transposes / casts / PSUM evacuations.
- phase A1 (all tiles): QKV matmul, sum-of-squares, evacuate qkv to SBUF bf16.
  Then ONE batched Ln + Exp(-0.5) pair computes every L2-norm rsqrt at once:
  the ACT table-set chooser is greedy (exp->exp_and_others, ln->natural_log),
  so interleaving Ln with Exp per-tile would thrash ACT_TABLE_LOADs (~2.7us
  each). Batching leaves 3 table loads total for the whole kernel.
- phase A2/B/C interleaved per-tile: normalize+RoPE+transposes, then banded
  attention, with the output projection for each 512-chunk as it completes.
- softmax denominators via DVE reciprocal_approx_fast (no ACT table), then
  gpsimd partition_broadcast; normalize reads only one PSUM operand.
- sliding-window masks are 3 constant 128x128 tiles (diag triangle, strict
  complement, and the t=2 strict|global variant); the kt==t-1 k-tile is fully
  unmasked and global rows for t>=3 are fully unmasked (no mask DMA stream).
- RoPE as 4 tensor_tensor ops using host-packed [cos|sin] and [sin|cos] tables.
"""

import sys

for _p in (
    "/opt/trn_rl_repo",
    "/root/.axon_site",
    "/root/.axon_site/_ro/pypackages",
    "/root/.axon_site/_ro/trn_rl_repo",
):
    if _p not in sys.path:
        sys.path.insert(0, _p)

from contextlib import ExitStack

import numpy as np

import concourse.bass as bass  # noqa: F401  (registers engine classes)
import concourse.tile as tile
from concourse import bacc, mybir
from concourse.bass_utils import run_bass_kernel_spmd
from concourse.masks import make_identity

B, S, DM = 2, 2048, 1024
NH, NKV, DH = 16, 4, 64
HPC = 4  # q heads per core (one full GQA group)
WINDOW, NGLOB = 256, 4
SCALE = 1.0 / np.sqrt(DH)
CAP = 15.0
EPS = 1e-8
P = 128
NT = S // P  # 16 sequence tiles
G = HPC + 1  # 4 q heads + 1 k head share L2norm/RoPE processing
F32 = mybir.dt.float32
BF16 = mybir.dt.bfloat16
MULT = mybir.AluOpType.mult
AF = mybir.ActivationFunctionType


def _build_kernel(ctx, tc, d):
    nc = tc.nc

    # weights first: the first QKV matmul only needs wqkv + xt[0], so those
    # DMAs go ahead of everything else (wo is 0.5 MB and not needed till C).
    consts = ctx.enter_context(tc.tile_pool(name="consts", bufs=1))
    wqkv_sb = consts.tile([P, 8, 384], BF16)
    nc.sync.dma_start(wqkv_sb[:], d["wqkv"].rearrange("(c p) n -> p c n", p=P))
    wo_sb = consts.tile([P, 2, DM], BF16)
    cs1_sb = consts.tile([P, NT, 64], BF16)
    cs2_sb = consts.tile([P, NT, 64], BF16)
    masks_sb = consts.tile([P, 3, P], BF16)
    ident = consts.tile([P, P], F32)
    ident_bf = consts.tile([P, P], BF16)

    def late_consts():
        nc.sync.dma_start(cs1_sb[:], d["cs1"].rearrange("(t p) n -> p t n", p=P))
        nc.sync.dma_start(cs2_sb[:], d["cs2"].rearrange("(t p) n -> p t n", p=P))
        nc.sync.dma_start(masks_sb[:], d["masks"].rearrange("p (j q) -> p j q", j=3))
        nc.sync.dma_start(wo_sb[:], d["wo"].rearrange("(c p) n -> p c n", p=P))
        make_identity(nc, ident[:])
        nc.vector.tensor_copy(ident_bf[:], ident[:])

    # persistent tensors
    qkv_pool = ctx.enter_context(tc.tile_pool(name="qkv", bufs=NT))
    qt_pool = ctx.enter_context(tc.tile_pool(name="qt", bufs=NT))
    kt_pool = ctx.enter_context(tc.tile_pool(name="kt", bufs=NT))
    ctx_pool = ctx.enter_context(tc.tile_pool(name="ctx", bufs=8))
    norm_pool = ctx.enter_context(tc.tile_pool(name="norm", bufs=1))

    xt_pool = ctx.enter_context(tc.tile_pool(name="xt", bufs=4))
    work = ctx.enter_context(tc.tile_pool(name="work", bufs=8))
    attn = ctx.enter_context(tc.tile_pool(name="attn", bufs=12))
    outp = ctx.enter_context(tc.tile_pool(name="outp", bufs=4))

    ps_t = ctx.enter_context(tc.tile_pool(name="ps_t", bufs=1, space="PSUM"))
    ps_mm = ctx.enter_context(tc.tile_pool(name="ps_mm", bufs=2, space="PSUM"))
    ps_sc = ctx.enter_context(tc.tile_pool(name="ps_sc", bufs=3, space="PSUM"))
    ps_cx = ctx.enter_context(tc.tile_pool(name="ps_cx", bufs=2, space="PSUM"))

    red_all = norm_pool.tile([P, NT, G], F32)
    lgs_all = norm_pool.tile([P, NT, G], F32)
    rcn_all = norm_pool.tile([P, NT, G], BF16)

    qkvtiles, qtiles, ktiles = [], [], []
    ctxt = [[None] * 4, [None] * 4]
    for c in range(2):
        for sc in range(4):
            ctile = ctx_pool.tile([P, 512], BF16, name=f"ctx_{c}_{sc}", tag="ctx")
            ctxt[c][sc] = ctile

    def phase_a1(i):
        xt = xt_pool.tile([P, 8, P], BF16, name=f"xt_{i}", tag="xt")
        nc.sync.dma_start(
            xt[:],
            d["xT"][:, P * i : P * (i + 1)].rearrange("(c p) s -> p c s", p=P),
        )
        pq = ps_mm.tile([P, 384], F32, name=f"pqkv_{i}", tag="mm")
        for mj in range(8):
            nc.tensor.matmul(
                pq[:],
                lhsT=xt[:, mj, :],
                rhs=wqkv_sb[:, mj, :],
                start=(mj == 0),
                stop=(mj == 7),
            )
        # sum of squares per head group (for the batched rsqrt later)
        ssq = work.tile([P, G * DH], F32, tag="ssq")
        nc.scalar.activation(ssq[:], pq[:, 0 : G * DH], AF.Square)
        nc.vector.tensor_reduce(
            red_all[:, i, :],
            ssq[:].rearrange("p (g n) -> p g n", g=G),
            axis=mybir.AxisListType.X,
            op=mybir.AluOpType.add,
        )
        # evacuate qkv to SBUF (bf16); col 384 is the ones column for the
        # softmax-denominator trick (v is cols 320:385 as the ctx lhsT).
        qkv_i = qkv_pool.tile([P, 385], BF16, name=f"qkv_{i}", tag="qkv")
        nc.scalar.copy(qkv_i[:, 0:384], pq[:])
        nc.vector.memset(qkv_i[:, 384:385], 1.0)
        qkvtiles.append(qkv_i)

    def norm_barrier(t0, t1):
        # batched Ln + Exp(-0.5 * .) over tiles [t0, t1): rsqrt(x) =
        # exp(-0.5*ln(x)). Each call costs 2 ACT table switches.
        nc.scalar.activation(lgs_all[:, t0:t1, :], red_all[:, t0:t1, :], AF.Ln)
        nc.scalar.activation(
            rcn_all[:, t0:t1, :], lgs_all[:, t0:t1, :], AF.Exp, scale=-0.5
        )

    def phase_a2(i):
        qkv_i = qkvtiles[i]
        qkn = work.tile([P, G * DH], BF16, tag="qkn")
        nc.vector.tensor_tensor(
            qkn[:].rearrange("p (g n) -> p g n", g=G),
            qkv_i[:, 0 : G * DH].rearrange("p (g n) -> p g n", g=G),
            rcn_all[:, i, :].unsqueeze(-1).broadcast_to([P, G, DH]),
            op=MULT,
        )

        # RoPE: rp[.., 0:32] = a*cos - b*sin ; rp[.., 32:64] = a*sin + b*cos
        qv = qkn[:].rearrange("p (g n) -> p g n", g=G)
        t_ac = work.tile([P, G * DH], BF16, tag="tac")
        t_as = work.tile([P, G * DH], BF16, tag="tas")
        nc.vector.tensor_tensor(
            t_ac[:].rearrange("p (g n) -> p g n", g=G),
            qv,
            cs1_sb[:, i, :].unsqueeze(1).broadcast_to([P, G, DH]),
            op=MULT,
        )
        nc.vector.tensor_tensor(
            t_as[:].rearrange("p (g n) -> p g n", g=G),
            qv,
            cs2_sb[:, i, :].unsqueeze(1).broadcast_to([P, G, DH]),
            op=MULT,
        )
        rp = work.tile([P, G * DH], BF16, tag="rp")
        rv = rp[:].rearrange("p (g n) -> p g n", g=G)
        acv = t_ac[:].rearrange("p (g n) -> p g n", g=G)
        asv = t_as[:].rearrange("p (g n) -> p g n", g=G)
        nc.vector.tensor_sub(rv[:, :, 0:32], acv[:, :, 0:32], acv[:, :, 32:64])
        nc.vector.tensor_add(rv[:, :, 32:64], asv[:, :, 0:32], asv[:, :, 32:64])

        # transpose q (2x 128-col blocks = 4 heads) and k (64 cols)
        qt_i = qt_pool.tile([64, HPC * P], BF16, name=f"qt_{i}", tag="qt")
        for hp in range(2):
            ptq = ps_t.tile([P, P], BF16, name=f"ptq_{i}_{hp}", tag="t")
            nc.tensor.transpose(ptq[:], rp[:, P * hp : P * (hp + 1)], ident_bf[:])
            nc.scalar.copy(qt_i[:, (2 * hp) * P : (2 * hp) * P + P], ptq[0:64, :])
            nc.vector.tensor_copy(
                qt_i[:, (2 * hp + 1) * P : (2 * hp + 1) * P + P], ptq[64:128, :]
            )
        ptk = ps_t.tile([P, P], BF16, name=f"ptk_{i}", tag="t")
        nc.tensor.transpose(ptk[0:64, :], rp[:, 256:320], ident_bf[:])
        kt_i = kt_pool.tile([64, P], BF16, name=f"kt_{i}", tag="kt")
        nc.scalar.copy(kt_i[:], ptk[0:64, :])
        qtiles.append(qt_i)
        ktiles.append(kt_i)

    def phase_b(t):
        kts = list(range(max(0, t - 2), t + 1))
        qrhs = qtiles[t][:].rearrange("p (h q) -> p h q", h=HPC)
        pcx = ps_cx.tile([65, 512], F32, name=f"pcx_{t}", tag="cx")
        n_ctx = len(kts) + (1 if t >= 3 else 0)

        # 1) all score matmuls first (keeps the PE stream dense)
        pss = []
        for kt in kts:
            ps = ps_sc.tile([P, 512], F32, name=f"psc_{t}_{kt}", tag="sc")
            nc.tensor.matmul(
                ps[:], lhsT=ktiles[kt][:], rhs=qrhs, start=True, stop=True
            )
            pss.append(ps)
        if t >= 3:
            # global rows (k < 4): fully unmasked for t >= 3
            psg = ps_sc.tile([4, 512], F32, name=f"psg_{t}", tag="sc")
            nc.tensor.matmul(
                psg[:], lhsT=ktiles[0][:, 0:4], rhs=qrhs, start=True, stop=True
            )

        # 2) exps (+ window masks)
        rhs_tiles = []
        for kt, ps in zip(kts, pss):
            ex = attn.tile([P, 512], BF16, tag="ex")
            nc.scalar.activation(ex[:], ps[:], AF.Exp, scale=SCALE)
            if kt == t:
                mk = 0  # diagonal: p <= q
            elif kt == t - 1:
                mk = None  # fully inside the window: no mask
            elif t == 2 and kt == 0:
                mk = 2  # strict complement + global rows
            else:
                mk = 1  # strict complement: p > q
            if mk is not None:
                em = attn.tile([P, 512], BF16, tag="em")
                nc.vector.tensor_tensor(
                    em[:].rearrange("p (h q) -> p h q", h=HPC),
                    ex[:].rearrange("p (h q) -> p h q", h=HPC),
                    masks_sb[:, mk, :].unsqueeze(1).broadcast_to([P, HPC, P]),
                    op=MULT,
                )
                rhs_tiles.append(em)
            else:
                rhs_tiles.append(ex)
        if t >= 3:
            exg = attn.tile([4, 512], BF16, tag="exg")
            nc.scalar.activation(exg[:], psg[:], AF.Exp, scale=SCALE)

        # 3) context matmuls (accumulate into pcx)
        ci = 0
        for kt, rhs_t in zip(kts, rhs_tiles):
            nc.tensor.matmul(
                pcx[:],
                lhsT=qkvtiles[kt][:, 320:385],
                rhs=rhs_t[:],
                start=(ci == 0),
                stop=(ci == n_ctx - 1),
            )
            ci += 1
        if t >= 3:
            nc.tensor.matmul(
                pcx[:],
                lhsT=qkvtiles[0][0:4, 320:385],
                rhs=exg[:],
                start=False,
                stop=True,
            )

        # softmax denominators (row 64 of pcx): reciprocal on DVE (no ACT
        # table), broadcast to 64 partitions on gpsimd so the normalize TT
        # reads only one PSUM operand (pcx).
        dn = attn.tile([1, 512], F32, tag="dn")
        nc.scalar.copy(dn[:], pcx[64:65, :])
        rcb = attn.tile([1, 512], F32, tag="rcb")
        nc.vector.reciprocal_approx_fast(rcb[:], dn[:])
        rb = attn.tile([64, 512], F32, tag="rb")
        nc.gpsimd.partition_broadcast(rb[:], rcb[:])

        sc_, qoff = t // 4, (t % 4) * P
        for h in range(HPC):
            c, p0 = h // 2, 64 * (h % 2)
            nc.vector.tensor_tensor(
                ctxt[c][sc_][p0 : p0 + 64, qoff : qoff + P],
                pcx[0:64, h * P : (h + 1) * P],
                rb[:, h * P : (h + 1) * P],
                op=MULT,
            )

    def phase_c_part(sc, part):
        # two mo slices per part: spreads each chunk's 16 output matmuls
        # across the following tiles as steady PE filler work.
        for mo in (2 * part, 2 * part + 1):
            po = ps_mm.tile([P, 512], F32, name=f"po_{sc}_{mo}", tag="mm")
            for c in range(2):
                nc.tensor.matmul(
                    po[:],
                    lhsT=wo_sb[:, c, P * mo : P * (mo + 1)],
                    rhs=ctxt[c][sc][:],
                    start=(c == 0),
                    stop=(c == 1),
                )
            ob = outp.tile([P, 512], BF16, tag="ob")
            if mo % 2 == 0:
                nc.scalar.copy(ob[:], po[:])
            else:
                nc.vector.tensor_copy(ob[:], po[:])
            nc.sync.dma_start(
                d["outT"][P * mo : P * (mo + 1), 512 * sc : 512 * (sc + 1)], ob[:]
            )

    # A1 for the first half, then its norm barrier; the second half of A1's
    # dense matmul stream is interleaved under the first B tiles so the PE
    # never sees a long idle window (HAM re-throttle) around the barrier.
    for t in range(8):
        phase_a1(t)
    late_consts()
    norm_barrier(0, 8)
    phase_a2(0)
    phase_a2(1)
    c_queue = []
    for t in range(NT):
        if t < 4:
            phase_a1(8 + 2 * t)
            phase_a1(9 + 2 * t)
        if t == 4:
            norm_barrier(8, NT)
        if t + 2 < NT:
            phase_a2(t + 2)
        phase_b(t)
        if t % 4 == 3:
            c_queue += [(t // 4, part) for part in range(4)]
        n_pop = len(c_queue) if t == NT - 1 else 1
        for _ in range(n_pop):
            if c_queue:
                phase_c_part(*c_queue.pop(0))


def build_program():
    nc = bacc.Bacc("TRN2", target_bir_lowering=False, debug=False, num_devices=8)
    d = {}
    d["xT"] = nc.dram_tensor("xT", [DM, S], BF16, kind="ExternalInput").ap()
    d["wqkv"] = nc.dram_tensor("wqkv", [DM, 384], BF16, kind="ExternalInput").ap()
    d["wo"] = nc.dram_tensor("wo", [256, DM], BF16, kind="ExternalInput").ap()
    d["cs1"] = nc.dram_tensor("cs1", [S, 64], BF16, kind="ExternalInput").ap()
    d["cs2"] = nc.dram_tensor("cs2", [S, 64], BF16, kind="ExternalInput").ap()
    d["masks"] = nc.dram_tensor("masks", [P, 3 * P], BF16, kind="ExternalInput").ap()
    d["outT"] = nc.dram_tensor("outT", [DM, S], BF16, kind="ExternalOutput").ap()
    with tile.TileContext(nc) as tc, ExitStack() as ctx:
        _build_kernel(ctx, tc, d)
    nc.compile()
    return nc


def make_masks(mask_np):
    """Build the 3 constant [k, q] mask tiles (diag tri, strict, t=2 variant)
    from the caller mask combined with the sliding-window|global pattern."""
    mask_np = np.asarray(mask_np).astype(bool)
    q = np.arange(S)[:, None]
    k = np.arange(S)[None, :]
    wmask = ((k <= q) & (k > q - WINDOW)) | (k < NGLOB)
    combT = (mask_np[0, 0] & wmask).T.astype(np.float32)  # [k, q]
    tri = combT[5 * P : 6 * P, 5 * P : 6 * P]  # t=5, kt=5 (diag)
    strict = combT[3 * P : 4 * P, 5 * P : 6 * P]  # t=5, kt=3 (strict)
    t2 = combT[0:P, 2 * P : 3 * P]  # t=2, kt=0 (strict | global)
    return np.stack([tri, strict, t2], axis=1)  # [P, 3, P]


def make_in_maps(x, cos, sin, mask, Wq, Wk, Wv, Wo):
    import ml_dtypes

    bf = ml_dtypes.bfloat16
    x = np.asarray(x, np.float32)
    cos = np.asarray(cos, np.float32)
    sin = np.asarray(sin, np.float32)
    Wq, Wk, Wv, Wo = (np.asarray(a, np.float32).astype(bf) for a in (Wq, Wk, Wv, Wo))
    cs1 = np.concatenate([cos, sin], axis=1).astype(bf)  # [S, 64]
    cs2 = np.concatenate([sin, cos], axis=1).astype(bf)
    masks = make_masks(mask).reshape(P, 3 * P).astype(bf)
    xT = [np.ascontiguousarray(x[b].T.astype(bf)) for b in range(B)]
    in_maps = []
    for c in range(8):
        b, g = divmod(c, 4)
        wqkv = np.concatenate(
            [
                Wq[:, 256 * g : 256 * (g + 1)],
                Wk[:, 64 * g : 64 * (g + 1)],
                Wv[:, 64 * g : 64 * (g + 1)],
            ],
            axis=1,
        )
        in_maps.append(
            {
                "xT": xT[b],
                "wqkv": np.ascontiguousarray(wqkv),
                "wo": np.ascontiguousarray(Wo[256 * g : 256 * (g + 1), :]),
                "cs1": cs1,
                "cs2": cs2,
                "masks": masks,
            }
        )
    return in_maps


_PROGRAM = None


def _get_program():
    global _PROGRAM
    if _PROGRAM is None:
        _PROGRAM = build_program()
    return _PROGRAM


def kernel(x, cos, sin, mask, Wq, Wk, Wv, Wo, _trace=False, _trace_kwargs=None):
    nc = _get_program()
    in_maps = make_in_maps(x, cos, sin, mask, Wq, Wk, Wv, Wo)
    res = run_bass_kernel_spmd(
        nc, in_maps, list(range(8)), trace=_trace, **(_trace_kwargs or {})
    )
    out = np.zeros((B, S, DM), np.float32)
    for c in range(8):
        out[c // 4] += np.asarray(res.results[c]["outT"], dtype=np.float32).T
    if _trace:
        kernel._last_results = res
    return out
